# revision 1
# baseline (speedup 1.0000x reference)
"""Trainium2 Bass kernel for nn_CausalCrossConditionalSelfAttention.

Strategy (8 NeuronCores, data-parallel over batch B=8, one element/core):
  - Host permutes tokens to interleaved temporal order => causal mask becomes
    lower-triangular, local mask becomes a narrow band (+2 prefix cols).
  - On-chip: qT/kT computed transposed [head*64, T]; v computed [T, head*65]
    with a ones column appended per head so each attention*V matmul also
    produces the softmax denominator Z as output row 64 (free Z reduction).
  - Scores computed transposed S^T[j, i] (keys on partitions) in 128x384
    blocks; fully-masked blocks skipped, fully-unmasked blocks unmasked,
    partial blocks multiplied by host-precomputed 0/1 mask tiles post-exp.
  - Conditional CLIP-token bias folded into the exp() activation bias column.
  - softmax normalization deferred: y~ = P_unnorm @ [v|1]; y = y~[:64] * (1/Z)
    broadcast via gpsimd partition_broadcast; mix-head combination folded in.
  - b_value/b_proj folded into a constant host-side output shift.
  - Matmuls run as float32r (full PE rate at N>=256, ~fp32 precision).

Self-contained: only needs numpy + the installed concourse/bass stack.
"""

import sys

if "/opt/trn_rl_repo" not in sys.path:
    sys.path.insert(0, "/opt/trn_rl_repo")

import numpy as np

# ----------------------------------------------------------------------------
# problem constants (hardcoded per spec)
# ----------------------------------------------------------------------------
BLOCK = 512
RECEP = 4
N_HEAD = 8
EMBED = 512
HS = 64
T = 2 * BLOCK + 2          # 1026
TP = 1152                  # 9 * 128
W = 384                    # query-chunk width, 3 chunks
NIC = TP // W
NJB = TP // 128
NSM = 10                   # softmaxes: heads 0..7, ml0 (s=8), ml1 (s=9)
NCORES = 8

# softmax id -> (mask kind, q/k source, v head)
SM_INFO = [
    (0, "loc", "main", 0), (1, "loc", "main", 1),
    (2, "seq", "main", 2), (3, "seq", "main", 3),
    (4, "seq", "main", 4), (5, "seq", "main", 5),
    (6, "seq", "main", 6), (7, "seq", "main", 7),
    (8, "loc", "ml", 2), (9, "loc", "ml", 3),
]
# emission order: heavy causal heads first, band heads last
SM_ORDER = [2, 0, 3, 1, 4, 8, 5, 9, 6, 7]


# ----------------------------------------------------------------------------
# host-side plan construction
# ----------------------------------------------------------------------------
def build_perm():
    perm = np.zeros(T, dtype=np.int64)
    perm[0], perm[1] = 0, 1
    b = np.arange(BLOCK)
    perm[2 + 2 * b] = 2 + b
    perm[3 + 2 * b] = 2 + BLOCK + b
    inv = np.argsort(perm)
    return perm, inv


def build_masks_orig():
    to = np.concatenate([np.zeros(2), np.arange(BLOCK) * 2 + 1, np.arange(BLOCK) * 2 + 2])
    seq = to[None, :] <= to[:, None]
    qo = np.concatenate([np.arange(BLOCK) * 2 + 1 - 2 * RECEP + 1] * 2)
    ko = np.concatenate([np.arange(BLOCK) * 2 + 1] * 2)
    de = ko[None, :] < qo[:, None]
    loc = seq.copy()
    loc[2:, 2:] = loc[2:, 2:] & (~de)
    return seq, loc


def build_block_plan():
    perm, _ = build_perm()
    seq, loc = build_masks_orig()
    Ms = np.zeros((TP, TP), dtype=bool)
    Ml = np.zeros((TP, TP), dtype=bool)
    Ms[:T, :T] = seq[perm][:, perm]
    Ml[:T, :T] = loc[perm][:, perm]
    # padded query rows are don't-care: replicate last real query row so
    # blocks classify as 'full'; padded key columns stay masked.
    Ms[T:] = Ms[T - 1]
    Ml[T:] = Ml[T - 1]

    mask_tiles = []
    tile_index = {}

    def tile_id(tile):
        key = tile.tobytes()
        if key not in tile_index:
            tile_index[key] = len(mask_tiles)
            mask_tiles.append(tile)
        return tile_index[key]

    plans = {}
    for kind, M in (("seq", Ms), ("loc", Ml)):
        plan = []
        for ic in range(NIC):
            blocks = []
            for jb in range(NJB):
                sub = M[ic * W:(ic + 1) * W, jb * 128:(jb + 1) * 128].T  # [128, W]
                if not sub.any():
                    continue
                nz_rows = np.flatnonzero(sub.any(axis=1))
                if sub.all():
                    blocks.append((jb, "full", None))
                elif jb == 0 and nz_rows.max() <= 1 and sub[nz_rows].all():
                    blocks.append((jb, "prefix", int(nz_rows.max()) + 1))
                else:
                    zcols = np.flatnonzero(~sub.all(axis=0))
                    c0, c1 = int(zcols.min()), int(zcols.max()) + 1
                    mid = tile_id(sub[:, c0:c1].astype(np.float32).copy())
                    blocks.append((jb, "part", (mid, c0, c1)))
            plan.append(blocks)
        plans[kind] = plan
    # concatenate cropped masks along the free dim; record offsets
    offs, cat = [], []
    o = 0
    for t in mask_tiles:
        offs.append((o, t.shape[1]))
        cat.append(t)
        o += t.shape[1]
    maskcat = np.concatenate(cat, axis=1) if cat else np.zeros((128, 0), np.float32)
    return plans, (maskcat, offs)


def prep_core_inputs(x_b, cond_b, w):
    """Per-core input tensors (numpy fp32) for the bass kernel."""
    perm, _ = build_perm()
    scale = np.float32(1.0 / np.sqrt(HS))

    xT = np.zeros((EMBED, TP), dtype=np.float32)
    xT[:, :T] = x_b[perm].T

    f = np.float32
    wq = np.ascontiguousarray(w["w_query"].astype(f).T * scale)
    wk = np.ascontiguousarray(w["w_key"].astype(f).T)
    wv = np.ascontiguousarray(w["w_value"].astype(f).T)
    wp = np.ascontiguousarray(w["w_proj"].astype(f).T)
    wqml = np.ascontiguousarray(w["w_query_ml"].astype(f).T * scale)
    wkml = np.ascontiguousarray(w["w_key_ml"].astype(f).T)

    bq = np.ascontiguousarray((w["b_query"].astype(f) * scale).reshape(4, 128).T)
    bk = np.ascontiguousarray(w["b_key"].astype(f).reshape(4, 128).T)
    bqml = (w["b_query_ml"].astype(f) * scale).reshape(128, 1).copy()
    bkml = w["b_key_ml"].astype(f).reshape(128, 1).copy()

    clip8 = np.maximum(w["att_bias_clip"].astype(f)[0, :, 0], 0.0) * 10.0
    clip2 = np.maximum(w["att_bias_clip_ml"].astype(f)[0, :, 0], 0.0) * 10.0
    biascols = np.zeros((128, NSM), dtype=f)
    if cond_b > 0:
        biascols[1, :N_HEAD] = clip8
        biascols[1, N_HEAD:] = clip2

    wg = w["w_mix"].astype(f)[:, 0, 0, 0]
    wl = w["w_mix"].astype(f)[:, 1, 0, 0]
    mixcol_s = np.ones(NSM, dtype=f)
    mixcol_s[2], mixcol_s[3] = wg[0], wg[1]
    mixcol_s[8], mixcol_s[9] = wl[0], wl[1]
    mixcol = mixcol_s[np.array(SM_ORDER)].reshape(NSM, 1).copy()

    return dict(xT=xT, wq=wq, wk=wk, wv=wv, wp=wp, wqml=wqml, wkml=wkml,
                bq=bq, bk=bk, bqml=bqml, bkml=bkml,
                biascols=biascols, mixcol=mixcol,
                ones8=np.ones((128, N_HEAD), dtype=f),
                onesrow=np.ones((1, 128), dtype=f))


def host_const_shift(w):
    bv = w["b_value"].astype(np.float64)
    wg = w["w_mix"].astype(np.float64)[:, 0, 0, 0]
    wl = w["w_mix"].astype(np.float64)[:, 1, 0, 0]
    scale_h = np.ones(N_HEAD)
    scale_h[2] = wg[0] + wl[0]
    scale_h[3] = wg[1] + wl[1]
    yshift = (bv.reshape(N_HEAD, HS) * scale_h[:, None]).reshape(-1)
    return (yshift @ w["w_proj"].astype(np.float64).T
            + w["b_proj"].astype(np.float64)).astype(np.float32)


# ----------------------------------------------------------------------------
# bass kernel emission
# ----------------------------------------------------------------------------
def emit_kernel(tc, ins, out_ap, plans, n_masks):
    from contextlib import ExitStack
    from concourse import mybir

    nc = tc.nc
    f32 = mybir.dt.float32
    f32r = mybir.dt.float32r
    AF = mybir.ActivationFunctionType

    def r(ap):
        return ap.bitcast(f32r)

    with ExitStack() as ctx:
        P = ctx.enter_context(tc.tile_pool(name="persist", bufs=1))
        xpool = ctx.enter_context(tc.tile_pool(name="xp", bufs=1))
        xT = [xpool.tile([128, TP], f32, name=f"x{k}", tag=f"x{k}") for k in range(4)]

        def loadw(name, ap, kchunks, ncols, eng=None):
            eng = eng or nc.sync
            tiles = []
            for kc in range(kchunks):
                t = P.tile([128, ncols], f32, name=f"{name}{kc}", tag=f"{name}{kc}")
                eng.dma_start(r(t[:]), r(ap[kc * 128:(kc + 1) * 128, :]))
                tiles.append(t)
            return tiles

        def loadw1(name, ap, kc, ncols, eng):
            t = P.tile([128, ncols], f32, name=f"{name}{kc}", tag=f"{name}{kc}")
            eng.dma_start(r(t[:]), r(ap[kc * 128:(kc + 1) * 128, :]))
            return t

        # interleave x chunks with the weight chunks each projection matmul
        # needs first, split across the two HWDGE queues
        nc.sync.dma_start(r(xT[0][:]), r(ins["xT"][0:128, :]))
        nc.scalar.dma_start(r(xT[2][:]), r(ins["xT"][256:384, :]))
        wq_sb = [loadw1("wq", ins["wq"], 0, 512, nc.sync)]
        wk_sb = [loadw1("wk", ins["wk"], 0, 512, nc.scalar)]
        nc.sync.dma_start(r(xT[1][:]), r(ins["xT"][128:256, :]))
        nc.scalar.dma_start(r(xT[3][:]), r(ins["xT"][384:512, :]))
        for kc in range(1, 4):
            wq_sb.append(loadw1("wq", ins["wq"], kc, 512, nc.sync))
            wk_sb.append(loadw1("wk", ins["wk"], kc, 512, nc.scalar))
        wv_sb = loadw("wv", ins["wv"], 4, 512)
        wqml_sb = loadw("wqml", ins["wqml"], 4, 128, nc.scalar)
        wkml_sb = loadw("wkml", ins["wkml"], 4, 128, nc.scalar)

        def loads(name, shape):
            t = P.tile(list(shape), f32, name=name, tag=name)
            nc.sync.dma_start(t[:], ins[name][:, :])
            return t

        bq_sb = loads("bq", (128, 4))
        bk_sb = loads("bk", (128, 4))
        bqml_sb = loads("bqml", (128, 1))
        bkml_sb = loads("bkml", (128, 1))
        biascols_sb = loads("biascols", (128, NSM))
        mixcol_sb = loads("mixcol", (NSM, 1))
        ins_onesrow = P.tile([1, 128], f32, name="onesrow", tag="onesrow")
        nc.sync.dma_start(r(ins_onesrow[:]), r(ins["onesrow"][:, :]))



        # persistent compute tiles
        qT = [P.tile([128, TP], f32, name=f"qT{m}", tag=f"qT{m}") for m in range(4)]
        kT = [P.tile([128, TP], f32, name=f"kT{m}", tag=f"kT{m}") for m in range(4)]
        qml = P.tile([128, TP], f32, name="qml", tag="qml")
        kml = P.tile([128, TP], f32, name="kml", tag="kml")
        vext = [P.tile([128, N_HEAD * 65], f32, name=f"vext{t}", tag=f"vext{t}")
                for t in range(NJB)]
        yTn = [P.tile([128, TP], f32, name=f"yTn{p}", tag=f"yTn{p}") for p in range(4)]
        tmpml = P.tile([128, TP], f32, name="tmpml", tag="tmpml")
        zall = P.tile([NSM, TP], f32, name="zall", tag="zall")
        rall = P.tile([NSM, TP], f32, name="rall", tag="rall")
        zorder = {s: i for i, s in enumerate(SM_ORDER)}

        if globals().get("DEBUG_TILES"):
            global _LAST_TILES
            _LAST_TILES = dict(qT=qT, kT=kT, qml=qml, kml=kml, vext=vext,
                               yTn=yTn, tmpml=tmpml, zall=zall)

        ptp = ctx.enter_context(tc.tile_pool(name="ptp", bufs=6))
        ostage = ctx.enter_context(tc.tile_pool(name="ostage", bufs=2))
        ztp = ctx.enter_context(tc.tile_pool(name="ztp", bufs=2))

        # ---------------- phase 1: projections ----------------
        with tc.tile_pool(name="pps", bufs=2, space="PSUM") as pps, \
             tc.tile_pool(name="vps", bufs=2, space="PSUM") as vps:

            def proj_T(wtiles, bias, dst_tiles, mchunks):
                # dst[c_out, t] = sum_c w[c, c_out] x[c, t] (+ bias[c_out]);
                # all 3 query chunks accumulate into one 3-bank psum group so
                # the evacuation is a single wide ACT op.
                for m in range(mchunks):
                    dst = dst_tiles[m] if mchunks > 1 else dst_tiles[0]
                    ps = pps.tile([128, 1536], f32, name="pp", tag="pp")
                    for ic in range(NIC):
                        for kc in range(4):
                            nc.tensor.matmul(
                                ps[:, ic * 512:ic * 512 + W],
                                lhsT=r(wtiles[kc][:, m * 128:(m + 1) * 128]),
                                rhs=r(xT[kc][:, ic * W:(ic + 1) * W]),
                                start=(kc == 0), stop=(kc == 3))
                    nc.vector.tensor_scalar_add(
                        r(dst[:].rearrange("p (g w) -> p g w", w=W)),
                        ps[:].rearrange("p (g c) -> p g c", c=512)[:, :, 0:W],
                        bias[:, m:m + 1])

            proj_T(wq_sb, bq_sb, qT, 4)
            proj_T(wk_sb, bk_sb, kT, 4)
            proj_T(wqml_sb, bqml_sb, [qml], 1)
            proj_T(wkml_sb, bkml_sb, [kml], 1)

            # v~ [t, 8*65] with ones col per head (ones via DMA)
            for tt in range(NJB):
                ps = vps.tile([128, 512], f32, name="vp", tag="vp")
                for kc in range(4):
                    nc.tensor.matmul(
                        ps[:],
                        lhsT=r(xT[kc][:, tt * 128:(tt + 1) * 128]),
                        rhs=r(wv_sb[kc][:]),
                        start=(kc == 0), stop=(kc == 3))
                vx = vext[tt][:].rearrange("p (h e) -> p h e", e=65)
                nc.scalar.activation(
                    r(vx[:, :, 0:64]), ps[:].rearrange("p (h d) -> p h d", d=64),
                    AF.Copy)
                nc.sync.dma_start(r(vx[:, :, 64:65]),
                                  r(ins["ones8"][:, :, None]))

        # late loads: wp (phase 4) and masks (phase 2 partial blocks)
        mask_offs = n_masks[1]
        maskw = n_masks[0]
        maskcat_sb = P.tile([128, maskw], f32, name="maskcat", tag="maskcat")
        nc.scalar.dma_start(maskcat_sb[:], ins["masks"][:, :])
        wp_sb = loadw("wp", ins["wp"], 4, 512)

        # ---------------- phase 2: attention softmaxes ----------------
        with tc.tile_pool(name="spsum", bufs=5, space="PSUM") as spsum, \
             tc.tile_pool(name="jps", bufs=1, space="PSUM") as jps, \
             tc.tile_pool(name="ypsum", bufs=2, space="PSUM") as ypsum:
            for s in SM_ORDER:
                _, kindname, src_, hv = SM_INFO[s]
                if src_ == "main":
                    qt, kt, off = qT[s // 2], kT[s // 2], (s % 2) * 64
                else:
                    qt, kt, off = qml, kml, (s - N_HEAD) * 64
                plan = plans[kindname]
                for ic in range(NIC):
                    i0 = ic * W
                    blocks = plan[ic]
                    Y = ypsum.tile([128, 512], f32, name="y", tag="y")
                    n_av = len(blocks)
                    avi = 0

                    def av(pt_ap, jb, rows=128):
                        nonlocal avi
                        nc.tensor.matmul(
                            Y[0:65, :W],
                            lhsT=r(vext[jb][0:rows, hv * 65:hv * 65 + 65]),
                            rhs=r(pt_ap),
                            start=(avi == 0), stop=(avi == n_av - 1))
                        avi += 1

                    units = [("one", [blk]) if (blk[0] == 0 or blk[1] == "prefix")
                             else ("pair", [blk]) for blk in blocks]

                    for kind_u, blks in units:
                        if kind_u == "one":
                            jb, bt, aux = blks[0]
                            ps = jps.tile([128, 512], f32, name="jp", tag="jp")
                            rows = aux if bt == "prefix" else 128
                            nc.tensor.matmul(
                                ps[0:rows, :W],
                                lhsT=r(kt[off:off + 64, 0:rows]) if bt == "prefix"
                                else r(kt[off:off + 64, 0:128]),
                                rhs=r(qt[off:off + 64, i0:i0 + W]),
                                start=True, stop=True)
                            pt = ptp.tile([128, W], f32, name="pt0", tag="pt0", bufs=3)
                            nc.scalar.activation(
                                r(pt[0:rows, :]), ps[0:rows, :W], AF.Exp,
                                bias=biascols_sb[0:rows, s:s + 1], scale=1.0)
                            if bt == "part":
                                mid, c0, c1 = aux
                                mo, mw = mask_offs[mid]
                                eng = nc.vector if kindname == "seq" else nc.gpsimd
                                eng.tensor_mul(r(pt[:, c0:c1]), pt[:, c0:c1],
                                               maskcat_sb[:, mo:mo + mw])
                            av(pt[0:rows, :], jb, rows)
                        else:
                            ps = spsum.tile([128, 512], f32, name="sp", tag="sp")
                            for g, (jb, bt, aux) in enumerate(blks):
                                nc.tensor.matmul(
                                    ps[:, g * 512:g * 512 + W],
                                    lhsT=r(kt[off:off + 64, jb * 128:jb * 128 + 128]),
                                    rhs=r(qt[off:off + 64, i0:i0 + W]),
                                    start=True, stop=True)
                            ng = len(blks)
                            pt = ptp.tile([128, ng * W], f32, name="pt", tag="pt")
                            nc.scalar.activation(
                                r(pt[:].rearrange("p (g w) -> p g w", w=W)),
                                ps[:].rearrange("p (g c) -> p g c", c=512)[:, 0:ng, 0:W],
                                AF.Exp)
                            for g, (jb, bt, aux) in enumerate(blks):
                                if bt == "part":
                                    mid, c0, c1 = aux
                                    mo, mw = mask_offs[mid]
                                    eng = nc.vector if kindname == "seq" else nc.gpsimd
                                    eng.tensor_mul(
                                        r(pt[:, g * W + c0:g * W + c1]),
                                        pt[:, g * W + c0:g * W + c1],
                                        maskcat_sb[:, mo:mo + mw])
                            for g, (jb, bt, aux) in enumerate(blks):
                                av(pt[:, g * W:(g + 1) * W], jb)

                    # evacuate unnormalized y (DVE) and Z row (DVE->DMA)
                    if s < N_HEAD:
                        dst = yTn[s // 2][(s % 2) * 64:(s % 2) * 64 + 64, i0:i0 + W]
                    else:
                        dst = tmpml[(s - N_HEAD) * 64:(s - N_HEAD) * 64 + 64, i0:i0 + W]
                    nc.vector.tensor_copy(r(dst), Y[0:64, :W])
                    zt = ztp.tile([1, W], f32, name="zt", tag="zt", bufs=2)
                    nc.vector.tensor_copy(zt[:], Y[64:65, :W])
                    zrow = zorder[s]
                    nc.sync.dma_start(zall[zrow:zrow + 1, i0:i0 + W], zt[:])
                # progressive reciprocal: rows [0:k] are final once the k-th
                # softmax in SM_ORDER is done (recomputing earlier rows is
                # idempotent - recip reads zall, writes rall)
                if SM_ORDER.index(s) in (3, 6):
                    k = SM_ORDER.index(s) + 1
                    nc.vector.reciprocal(rall[0:k, :], zall[0:k, :])
                    nc.vector.tensor_scalar_mul(rall[0:k, :], rall[0:k, :],
                                                mixcol_sb[0:k, :])

        # ---------------- phase 3+4: normalization + output projection ----
        nc.vector.reciprocal(rall[:], zall[:])
        nc.vector.tensor_scalar_mul(rall[:], rall[:], mixcol_sb[:])

        if globals().get("DEBUG_PRENORM") is not None:
            for _m in range(4):
                nc.sync.dma_start(DEBUG_PRENORM[_m], yTn[_m][:])
            nc.sync.dma_start(DEBUG_PRENORM[4], tmpml[:])

        with tc.tile_pool(name="rbps", bufs=2, space="PSUM") as rbps, \
             tc.tile_pool(name="opsum", bufs=2, space="PSUM") as opsum:
            _rbi = [0]

            def bcast_row(s_idx, eng):
                # recip row s -> partition-0 staging -> PE ones-matmul
                # broadcast into all 128 partitions of a 3-bank psum tile
                zr = ztp.tile([1, TP], f32, name="zr", tag="zr", bufs=2)
                row = zorder[s_idx]
                eng.dma_start(r(zr[:]), r(rall[row:row + 1, :]))
                rb = rbps.tile([128, 1536], f32, name="rb", tag="rb")
                for g in range(NIC):
                    nc.tensor.matmul(
                        rb[:, g * 512:g * 512 + W],
                        lhsT=r(ins_onesrow[0:1, :]),
                        rhs=r(zr[0:1, g * W:(g + 1) * W]),
                        start=True, stop=True)
                return rb

            def norm_pair(dsttile, sa, sb):
                rba = bcast_row(sa, nc.sync)
                rbb = bcast_row(sb, nc.scalar)
                for half, rb in ((0, rba), (1, rbb)):
                    p0 = half * 64
                    nc.vector.tensor_mul(
                        r(dsttile[p0:p0 + 64].rearrange("p (g w) -> p g w", w=W)),
                        dsttile[p0:p0 + 64].rearrange("p (g w) -> p g w", w=W),
                        rb[:].rearrange("p (g c) -> p g c", c=512)[p0:p0 + 64, :, 0:W])

            norm_pair(yTn[0], 0, 1)
            norm_pair(yTn[2], 4, 5)
            norm_pair(yTn[3], 6, 7)
            norm_pair(yTn[1], 2, 3)
            norm_pair(tmpml, 8, 9)
            nc.vector.tensor_add(r(yTn[1][:]), yTn[1][:], tmpml[:])
            for m in range(NJB):
                po = opsum.tile([128, 512], f32, name="po", tag="po")
                for p in range(4):
                    nc.tensor.matmul(
                        po[:],
                        lhsT=r(yTn[p][:, m * 128:(m + 1) * 128]),
                        rhs=r(wp_sb[p][:]),
                        start=(p == 0), stop=(p == 3))
                ost = ostage.tile([128, 512], f32, name="ost", tag="ost")
                nc.vector.tensor_copy(ost[:], po[:])
                eng = nc.sync if m % 2 == 0 else nc.scalar
                eng.dma_start(out_ap[m * 128:(m + 1) * 128, :], ost[:])


# ----------------------------------------------------------------------------
# module build + run
# ----------------------------------------------------------------------------
_CACHE = {}


def _get_module():
    if "nc" in _CACHE:
        return _CACHE["nc"], _CACHE["plans"], _CACHE["mask_tiles"]
    import concourse.tile as tile
    from concourse import bacc, mybir

    plans, (maskcat, mask_offs) = build_block_plan()

    nc = bacc.Bacc("TRN2", target_bir_lowering=False, debug=False,
                   enable_asserts=False, num_devices=NCORES)
    f32 = mybir.dt.float32

    def din(name, shape):
        return nc.dram_tensor(name, list(shape), f32, kind="ExternalInput").ap()

    ins = dict(
        xT=din("xT", (EMBED, TP)),
        wq=din("wq", (EMBED, EMBED)), wk=din("wk", (EMBED, EMBED)),
        wv=din("wv", (EMBED, EMBED)), wp=din("wp", (EMBED, EMBED)),
        wqml=din("wqml", (EMBED, 128)), wkml=din("wkml", (EMBED, 128)),
        bq=din("bq", (128, 4)), bk=din("bk", (128, 4)),
        bqml=din("bqml", (128, 1)), bkml=din("bkml", (128, 1)),
        biascols=din("biascols", (128, NSM)),
        mixcol=din("mixcol", (NSM, 1)),
        masks=din("masks", (128, maskcat.shape[1])),
        ones8=din("ones8", (128, N_HEAD)),
        onesrow=din("onesrow", (1, 128)),
    )
    out_ap = nc.dram_tensor("out_p", [TP, EMBED], f32, kind="ExternalOutput").ap()

    with tile.TileContext(nc) as tc:
        emit_kernel(tc, ins, out_ap, plans, (maskcat.shape[1], mask_offs))
    nc.compile()

    _CACHE.update(nc=nc, plans=plans, mask_tiles=maskcat)
    return nc, plans, maskcat


def build_in_maps(inputs):
    """Per-core input maps; weights/masks prepped once and shared."""
    nc, plans, maskcat = _get_module()
    x = inputs["x"].astype(np.float32)
    cond = np.asarray(inputs["cond_mask"]).astype(np.int32)
    B = x.shape[0]
    assert B == NCORES, f"expected B={NCORES}, got {B}"

    perm, _ = build_perm()
    base0 = prep_core_inputs(x[0], int(cond[0]), inputs)
    base0["masks"] = maskcat
    in_maps = [base0]
    for b in range(1, B):
        ci = dict(base0)
        xT = np.zeros((EMBED, TP), dtype=np.float32)
        xT[:, :T] = x[b][perm].T
        ci["xT"] = xT
        if int(cond[b]) != int(cond[0]):
            biascols = base0["biascols"].copy()
            if int(cond[b]) > 0:
                f = np.float32
                clip8 = np.maximum(inputs["att_bias_clip"].astype(f)[0, :, 0], 0.0) * 10.0
                clip2 = np.maximum(inputs["att_bias_clip_ml"].astype(f)[0, :, 0], 0.0) * 10.0
                biascols[1, :N_HEAD] = clip8
                biascols[1, N_HEAD:] = clip2
            else:
                biascols[:] = 0.0
            ci["biascols"] = biascols
        in_maps.append(ci)
    return nc, in_maps


def kernel(**inputs):
    from concourse import bass_utils

    inputs = {k: np.asarray(v) for k, v in inputs.items()}
    nc, in_maps = build_in_maps(inputs)
    res = bass_utils.run_bass_kernel_spmd(nc, in_maps, core_ids=list(range(NCORES)))
    _CACHE["last_results"] = res

    _, inv = build_perm()
    shift = host_const_shift(inputs)
    B = inputs["x"].shape[0]
    out = np.empty((B, T, EMBED), dtype=np.float32)
    for b in range(B):
        out[b] = res.results[b]["out_p"][:T][inv] + shift
    return out



# revision 5
# speedup vs baseline: 1.3482x; 1.3482x over previous
"""Trainium2 Bass kernel for nn_CausalCrossConditionalSelfAttention.

Strategy (8 NeuronCores, data-parallel over batch B=8, one element/core):
  - Host permutes tokens to interleaved temporal order => causal mask becomes
    lower-triangular, local mask becomes a narrow band (+2 prefix cols).
  - T=1026 everywhere (no padding): engine cost on TRN2 scales with the
    free-dim size only, so every block is cropped to its live query range
    [q0,q1) and live key range [klo,khi] - the causal upper triangle, the
    band, and the 2-token tail cost only what they cover.
  - qT/kT/v computed via fp32r projections (psum) evacuated to bf16; all
    attention matmuls run in bf16 (full PE rate at any free size).
  - Scores computed transposed S^T[k, q] (keys on partitions); exp on ACT
    with the conditional CLIP bias folded in as a per-partition bias column;
    partial blocks multiplied by host-precomputed 0/1 bf16 mask tiles (DVE
    2x mode).
  - v gets a ones column per head so attention*V also produces the softmax
    denominator Z (psum row 64); Z rows are evacuated by DMA (free engines).
  - Normalization: 1/Z per softmax-pair (DVE reciprocal, bf16), broadcast to
    128 partitions on gpsimd, multiplied into bf16 y tiles (DVE 2x); the
    head-mix weights are folded into the Y evacuation (tensor_scalar).
  - Emission is software-pipelined: projections for heads 2,3 first, then
    attention interleaved with the remaining projections so PE never idles
    and ACT exp work starts ~6us in.
  - b_value/b_proj folded into a constant host-side output shift.

Self-contained: only needs numpy + the installed concourse/bass stack.
"""

import sys

if "/opt/trn_rl_repo" not in sys.path:
    sys.path.insert(0, "/opt/trn_rl_repo")

import numpy as np
import ml_dtypes

BF16 = ml_dtypes.bfloat16

# ----------------------------------------------------------------------------
# problem constants (hardcoded per spec)
# ----------------------------------------------------------------------------
BLOCK = 512
RECEP = 4
N_HEAD = 8
EMBED = 512
HS = 64
T = 2 * BLOCK + 2          # 1026
QCH = [(0, 384), (384, 768), (768, T)]      # query chunks
NKB = (T + 127) // 128                       # 9 key blocks (last has 2 keys)
NSM = 10                   # softmaxes: heads 0..7, ml0 (s=8), ml1 (s=9)
NCORES = 8

# softmax id -> (mask kind, q/k source, v head)
SM_INFO = [
    (0, "loc", "main", 0), (1, "loc", "main", 1),
    (2, "seq", "main", 2), (3, "seq", "main", 3),
    (4, "seq", "main", 4), (5, "seq", "main", 5),
    (6, "seq", "main", 6), (7, "seq", "main", 7),
    (8, "loc", "ml", 2), (9, "loc", "ml", 3),
]
# emission order: softmaxes of a norm-pair adjacent; heavy heads early so the
# mix-add and output projection dependencies resolve before the tail
SM_ORDER = [2, 3, 8, 9, 4, 5, 6, 7, 0, 1]
ZROW = {s: i for i, s in enumerate(SM_ORDER)}


# ----------------------------------------------------------------------------
# host-side plan construction
# ----------------------------------------------------------------------------
def build_perm():
    perm = np.zeros(T, dtype=np.int64)
    perm[0], perm[1] = 0, 1
    b = np.arange(BLOCK)
    perm[2 + 2 * b] = 2 + b
    perm[3 + 2 * b] = 2 + BLOCK + b
    inv = np.argsort(perm)
    return perm, inv


def build_masks_orig():
    to = np.concatenate([np.zeros(2), np.arange(BLOCK) * 2 + 1, np.arange(BLOCK) * 2 + 2])
    seq = to[None, :] <= to[:, None]
    qo = np.concatenate([np.arange(BLOCK) * 2 + 1 - 2 * RECEP + 1] * 2)
    ko = np.concatenate([np.arange(BLOCK) * 2 + 1] * 2)
    de = ko[None, :] < qo[:, None]
    loc = seq.copy()
    loc[2:, 2:] = loc[2:, 2:] & (~de)
    return seq, loc


def build_block_plan():
    """Per (kind, query-chunk): list of cropped key-block descriptors."""
    perm, _ = build_perm()
    seq, loc = build_masks_orig()

    mask_tiles = []
    tile_index = {}

    def tile_id(tile):
        key = tile.tobytes() + str(tile.shape).encode()
        if key not in tile_index:
            tile_index[key] = len(mask_tiles)
            mask_tiles.append(tile)
        return tile_index[key]

    plans = {}
    for kind, M0 in (("seq", seq), ("loc", loc)):
        M = M0[perm][:, perm]                 # [T, T] query-major, permuted
        plan = []
        for (qc0, qc1) in QCH:
            blocks = []
            for jb in range(NKB):
                k0, k1 = jb * 128, min(jb * 128 + 128, T)
                sub = M[qc0:qc1, k0:k1]       # [q, k]
                if not sub.any():
                    continue
                qnz = np.flatnonzero(sub.any(axis=1))
                q0, q1 = int(qc0 + qnz.min()), int(qc0 + qnz.max() + 1)
                knz = np.flatnonzero(sub.any(axis=0))
                # PE requires operand base partitions equal (and in {0,32,64}),
                # so key-row cropping only trims the top (khi); klo stays 0 and
                # fully-masked low rows are zeroed by the mask multiply.
                klo, khi = 0, int(knz.max())
                subc = sub[q0 - qc0:q1 - qc0, klo:khi + 1].T  # [kc, f]
                if subc.all():
                    mask = None
                else:
                    bad = np.flatnonzero(~subc.all(axis=0))
                    c0, c1 = int(bad.min()), int(bad.max() + 1)
                    mid = tile_id(subc[:, c0:c1].astype(np.float32).copy())
                    mask = (mid, c0, c1)
                blocks.append(dict(jb=jb, q0=q0, q1=q1, klo=klo, khi=khi,
                                   mask=mask, bias=(jb == 0)))
            plan.append(blocks)
        plans[kind] = plan

    # concatenate cropped masks along the free dim; record offsets
    offs, cat = [], []
    o = 0
    maxp = max((t.shape[0] for t in mask_tiles), default=1)
    for t in mask_tiles:
        offs.append((o, t.shape[1]))
        tp = np.zeros((maxp, t.shape[1]), np.float32)
        tp[:t.shape[0]] = t
        cat.append(tp)
        o += t.shape[1]
    maskcat = (np.concatenate(cat, axis=1) if cat
               else np.zeros((1, 1), np.float32))
    return plans, (maskcat, offs)


def prep_shared_inputs(w):
    """Weight/mask tensors shared by all cores (numpy, fp32/bf16)."""
    scale = np.float32(1.0 / np.sqrt(HS))
    f = np.float32
    ins = dict(
        wq=np.ascontiguousarray(w["w_query"].astype(f).T * scale),
        wk=np.ascontiguousarray(w["w_key"].astype(f).T),
        wv=np.ascontiguousarray(w["w_value"].astype(f).T),
        wqml=np.ascontiguousarray(w["w_query_ml"].astype(f).T * scale),
        wkml=np.ascontiguousarray(w["w_key_ml"].astype(f).T),
        wp=np.ascontiguousarray(w["w_proj"].astype(f).T.astype(BF16)),
    )

    # packed per-partition smalls: bq(4) bk(4) bqml(1) bkml(1) biascols(10)
    # mixsc(10) -> [128, 30]; biascols filled per-core later
    smalls = np.zeros((128, 30), dtype=f)
    smalls[:, 0:4] = (w["b_query"].astype(f) * scale).reshape(4, 128).T
    smalls[:, 4:8] = w["b_key"].astype(f).reshape(4, 128).T
    smalls[:, 8] = w["b_query_ml"].astype(f) * scale
    smalls[:, 9] = w["b_key_ml"].astype(f)
    wg = w["w_mix"].astype(f)[:, 0, 0, 0]
    wl = w["w_mix"].astype(f)[:, 1, 0, 0]
    mixsc = np.ones(NSM, dtype=f)
    mixsc[2], mixsc[3] = wg[0], wg[1]
    mixsc[8], mixsc[9] = wl[0], wl[1]
    mixsc = np.where(np.abs(mixsc) < 1e-20, 1e-20, mixsc)
    ins["ones10"] = np.broadcast_to((1.0 / mixsc)[None, :], (128, NSM)).astype(BF16).copy()
    ins["smalls"] = smalls
    return ins


def biascols_for(w, cond_b):
    f = np.float32
    biascols = np.zeros((128, NSM), dtype=f)
    if cond_b > 0:
        clip8 = np.maximum(w["att_bias_clip"].astype(f)[0, :, 0], 0.0) * 10.0
        clip2 = np.maximum(w["att_bias_clip_ml"].astype(f)[0, :, 0], 0.0) * 10.0
        biascols[1, :N_HEAD] = clip8
        biascols[1, N_HEAD:] = clip2
    return biascols


def host_const_shift(w):
    bv = w["b_value"].astype(np.float64)
    wg = w["w_mix"].astype(np.float64)[:, 0, 0, 0]
    wl = w["w_mix"].astype(np.float64)[:, 1, 0, 0]
    scale_h = np.ones(N_HEAD)
    scale_h[2] = wg[0] + wl[0]
    scale_h[3] = wg[1] + wl[1]
    yshift = (bv.reshape(N_HEAD, HS) * scale_h[:, None]).reshape(-1)
    return (yshift @ w["w_proj"].astype(np.float64).T
            + w["b_proj"].astype(np.float64)).astype(np.float32)


# ----------------------------------------------------------------------------
# bass kernel emission
# ----------------------------------------------------------------------------
def emit_kernel(tc, ins, out_ap, plans, mask_meta):
    from contextlib import ExitStack
    from concourse import mybir

    nc = tc.nc
    f32 = mybir.dt.float32
    f32r = mybir.dt.float32r
    bf16 = mybir.dt.bfloat16
    AF = mybir.ActivationFunctionType

    def r(ap):
        return ap.bitcast(f32r)

    maskw, mask_offs = mask_meta

    with ExitStack() as ctx:
        P = ctx.enter_context(tc.tile_pool(name="persist", bufs=1))
        ptp = ctx.enter_context(tc.tile_pool(name="ptp", bufs=4))
        ostage = ctx.enter_context(tc.tile_pool(name="ostage", bufs=2))
        mm = ctx.enter_context(tc.tile_pool(name="mm", bufs=2, space="PSUM"))
        sp = ctx.enter_context(tc.tile_pool(name="sp", bufs=3, space="PSUM"))
        yp = ctx.enter_context(tc.tile_pool(name="yp", bufs=3, space="PSUM"))

        # ---------------- input DMA (two HWDGE queues) ----------------
        xT = [P.tile([128, T], f32, name=f"x{k}", tag=f"x{k}") for k in range(4)]
        wq_sb, wk_sb, wv_sb, wp_sb = [], [], [], []
        wqml_sb, wkml_sb = [], []

        def loadw1(lst, name, ap, kc, ncols, eng, dt=f32):
            t = P.tile([128, ncols], dt, name=f"{name}{kc}", tag=f"{name}{kc}")
            if dt is f32:
                eng.dma_start(r(t[:]), r(ap[kc * 128:(kc + 1) * 128, :]))
            else:
                eng.dma_start(t[:], ap[kc * 128:(kc + 1) * 128, :])
            lst.append(t)

        nc.sync.dma_start(r(xT[0][:]), r(ins["xT"][0:128, :]))
        nc.scalar.dma_start(r(xT[2][:]), r(ins["xT"][256:384, :]))
        loadw1(wq_sb, "wq", ins["wq"], 0, 512, nc.sync)
        loadw1(wk_sb, "wk", ins["wk"], 0, 512, nc.scalar)
        nc.sync.dma_start(r(xT[1][:]), r(ins["xT"][128:256, :]))
        nc.scalar.dma_start(r(xT[3][:]), r(ins["xT"][384:512, :]))
        for kc in range(1, 4):
            loadw1(wq_sb, "wq", ins["wq"], kc, 512, nc.sync)
            loadw1(wk_sb, "wk", ins["wk"], kc, 512, nc.scalar)
        smalls_sb = P.tile([128, 30], f32, name="smalls", tag="smalls")
        nc.scalar.dma_start(smalls_sb[:], ins["smalls"][:, :])
        maskcat_sb = P.tile([128, maskw], bf16, name="maskcat", tag="maskcat")
        nc.scalar.dma_start(maskcat_sb[:], ins["masks"][:, :])
        for kc in range(4):
            loadw1(wv_sb, "wv", ins["wv"], kc, 512, nc.sync)
        for kc in range(4):
            loadw1(wqml_sb, "wqml", ins["wqml"], kc, 128, nc.scalar)
            loadw1(wkml_sb, "wkml", ins["wkml"], kc, 128, nc.scalar)
        for kc in range(4):
            loadw1(wp_sb, "wp", ins["wp"], kc, 512, nc.sync, dt=bf16)

        bq = smalls_sb[:, 0:4]
        bk = smalls_sb[:, 4:8]
        bqml = smalls_sb[:, 8:9]
        bkml = smalls_sb[:, 9:10]
        biascols = smalls_sb[:, 10:20]

        # ---------------- persistent compute tiles ----------------
        qT = [P.tile([128, T], bf16, name=f"qT{m}", tag=f"qT{m}") for m in range(4)]
        kT = [P.tile([128, T], bf16, name=f"kT{m}", tag=f"kT{m}") for m in range(4)]
        qml = P.tile([128, T], bf16, name="qml", tag="qml")
        kml = P.tile([128, T], bf16, name="kml", tag="kml")
        vext = [P.tile([128, NSM * 65], bf16, name=f"vext{t}", tag=f"vext{t}")
                for t in range(NKB)]
        yTn = [P.tile([128, T], bf16, name=f"yTn{p}", tag=f"yTn{p}") for p in range(4)]
        tmpml = P.tile([128, T], bf16, name="tmpml", tag="tmpml")
        rzp = ctx.enter_context(tc.tile_pool(name="rzp", bufs=3))
        rbcp2 = ctx.enter_context(tc.tile_pool(name="rbcp2", bufs=3))

        # ones columns of vext = 1/mix per softmax (input-dependent, via DMA)
        for tt in range(NKB):
            rows = min(128, T - tt * 128)
            vx = vext[tt][0:rows].rearrange("p (s e) -> p s e", e=65)
            nc.sync.dma_start(vx[:, :, 64:65], ins["ones10"][0:rows, :, None])

        # ---------------- emission helpers ----------------
        def proj_chunk(wtiles, bias_ap, dst, m, evac_eng):
            """dst[m*128:(m+1)*128 rows of c_out, :] over 3 query chunks."""
            for ci, (a, b) in enumerate(QCH):
                ps = mm.tile([128, 512], f32, name="mm", tag="mm")
                for kc in range(4):
                    nc.tensor.matmul(
                        ps[:, 0:b - a],
                        lhsT=r(wtiles[kc][:, m * 128:(m + 1) * 128]),
                        rhs=r(xT[kc][:, a:b]),
                        start=(kc == 0), stop=(kc == 3))
                evac_eng(dst, ps, a, b, m)

        def vproj(tt):
            rows = min(128, T - tt * 128)
            ps = mm.tile([128, 512], f32, name="mm", tag="mm")
            for kc in range(4):
                nc.tensor.matmul(
                    ps[0:rows, :],
                    lhsT=r(xT[kc][:, tt * 128:tt * 128 + rows]),
                    rhs=r(wv_sb[kc][:]),
                    start=(kc == 0), stop=(kc == 3))
            vx = vext[tt][0:rows].rearrange("p (s e) -> p s e", e=65)
            nc.scalar.activation(vx[:, 0:8, 0:64],
                                 ps[0:rows].rearrange("p (h d) -> p h d", d=64),
                                 AF.Copy)
            nc.scalar.activation(vx[:, 8:10, 0:64],
                                 ps[0:rows, 128:256].rearrange("p (h d) -> p h d", d=64),
                                 AF.Copy)

        mask_engines = [nc.vector, nc.vector, nc.gpsimd]
        _mrr = [0]

        def attn(s, ic):
            _, kindname, src_, hv = SM_INFO[s]
            if src_ == "main":
                qt, kt, off = qT[s // 2], kT[s // 2], (s % 2) * 64
            else:
                qt, kt, off = qml, kml, (s - N_HEAD) * 64
            blocks = plans[kindname][ic]
            qc0, qc1 = QCH[ic]
            wc = qc1 - qc0
            Y = yp.tile([65, 512], f32, name="y", tag="y")
            nav = len(blocks)
            for bi, blk in enumerate(blocks):
                jb, q0, q1 = blk["jb"], blk["q0"], blk["q1"]
                klo, khi = blk["klo"], blk["khi"]
                kc = khi - klo + 1
                f = q1 - q0
                k0 = jb * 128
                ps = sp.tile([128, 512], f32, name="sp", tag="sp")
                nc.tensor.matmul(
                    ps[0:kc, 0:f],
                    lhsT=kt[off:off + 64, k0 + klo:k0 + khi + 1],
                    rhs=qt[off:off + 64, q0:q1],
                    start=True, stop=True)
                pt = ptp.tile([128, 512], bf16, name="pt", tag="pt")
                bias = biascols[klo:klo + kc, s:s + 1] if blk["bias"] else 0.0
                nc.scalar.activation(pt[0:kc, 0:f], ps[0:kc, 0:f], AF.Exp,
                                     bias=bias, scale=1.0)
                if blk["mask"] is not None:
                    mid, c0, c1 = blk["mask"]
                    mo, mw = mask_offs[mid]
                    eng = mask_engines[_mrr[0] % len(mask_engines)]
                    _mrr[0] += 1
                    eng.tensor_mul(pt[0:kc, c0:c1], pt[0:kc, c0:c1],
                                   maskcat_sb[0:kc, mo:mo + mw])
                nc.tensor.matmul(
                    Y[0:65, q0 - qc0:q1 - qc0],
                    lhsT=vext[jb][klo:klo + kc, s * 65:s * 65 + 65],
                    rhs=pt[0:kc, 0:f],
                    start=(bi == 0), stop=(bi == nav - 1))
            # fused evacuation + normalization: row 64 of Y is Z/mix
            if s < N_HEAD:
                dst = yTn[s // 2][(s % 2) * 64:(s % 2) * 64 + 64, qc0:qc1]
            else:
                dst = tmpml[(s - N_HEAD) * 64:(s - N_HEAD) * 64 + 64, qc0:qc1]
            rz = rzp.tile([1, 512], bf16, name="rz", tag="rz")
            with nc.allow_low_precision("1/Z in bf16; target relerr 2e-2"):
                nc.vector.reciprocal(rz[0:1, 0:wc], Y[64:65, 0:wc])
            rbc = rbcp2.tile([64, 512], bf16, name="rbc", tag="rbc")
            nc.gpsimd.partition_broadcast(rbc[0:64, 0:wc], rz[0:1, 0:wc],
                                          channels=64)
            nc.vector.tensor_mul(dst, Y[0:64, 0:wc], rbc[0:64, 0:wc])

        def attn_head(s):
            for ic in range(3):
                attn(s, ic)

        # ---------------- pipelined emission ----------------
        proj_chunk(wq_sb, None, qT[1], 1, lambda d, p, a, b, m: nc.scalar.activation(
            d[:, a:b], p[:, 0:b - a], AF.Identity, bias=bq[:, m:m + 1], scale=1.0))
        proj_chunk(wk_sb, None, kT[1], 1, lambda d, p, a, b, m: nc.scalar.activation(
            d[:, a:b], p[:, 0:b - a], AF.Identity, bias=bk[:, m:m + 1], scale=1.0))
        vproj(0), vproj(1), vproj(2)
        attn(2, 0)
        vproj(3), vproj(4), vproj(5)
        attn(2, 1)
        vproj(6), vproj(7), vproj(8)
        attn(2, 2)
        proj_chunk(wqml_sb, None, qml, 0, lambda d, p, a, b, m: nc.scalar.activation(
            d[:, a:b], p[:, 0:b - a], AF.Identity, bias=bqml[:, 0:1], scale=1.0))
        attn(3, 0)
        proj_chunk(wkml_sb, None, kml, 0, lambda d, p, a, b, m: nc.scalar.activation(
            d[:, a:b], p[:, 0:b - a], AF.Identity, bias=bkml[:, 0:1], scale=1.0))
        attn(3, 1)
        attn(3, 2)
        attn_head(8)
        proj_chunk(wq_sb, None, qT[2], 2, lambda d, p, a, b, m: nc.scalar.activation(
            d[:, a:b], p[:, 0:b - a], AF.Identity, bias=bq[:, m:m + 1], scale=1.0))
        attn_head(9)
        proj_chunk(wk_sb, None, kT[2], 2, lambda d, p, a, b, m: nc.scalar.activation(
            d[:, a:b], p[:, 0:b - a], AF.Identity, bias=bk[:, m:m + 1], scale=1.0))
        nc.vector.tensor_add(yTn[1][:], yTn[1][:], tmpml[:])
        attn_head(4)
        proj_chunk(wq_sb, None, qT[3], 3, lambda d, p, a, b, m: nc.scalar.activation(
            d[:, a:b], p[:, 0:b - a], AF.Identity, bias=bq[:, m:m + 1], scale=1.0))
        attn_head(5)
        proj_chunk(wk_sb, None, kT[3], 3, lambda d, p, a, b, m: nc.scalar.activation(
            d[:, a:b], p[:, 0:b - a], AF.Identity, bias=bk[:, m:m + 1], scale=1.0))
        attn_head(6)
        proj_chunk(wq_sb, None, qT[0], 0, lambda d, p, a, b, m: nc.scalar.activation(
            d[:, a:b], p[:, 0:b - a], AF.Identity, bias=bq[:, m:m + 1], scale=1.0))
        attn_head(7)
        proj_chunk(wk_sb, None, kT[0], 0, lambda d, p, a, b, m: nc.scalar.activation(
            d[:, a:b], p[:, 0:b - a], AF.Identity, bias=bk[:, m:m + 1], scale=1.0))
        attn_head(0)
        attn_head(1)

        # ---------------- output projection ----------------
        for mb in range(NKB):
            t0 = mb * 128
            rows = min(128, T - t0)
            po = mm.tile([128, 512], f32, name="mm", tag="mm")
            for p in range(4):
                nc.tensor.matmul(
                    po[0:rows, :],
                    lhsT=yTn[p][:, t0:t0 + rows],
                    rhs=wp_sb[p][:],
                    start=(p == 0), stop=(p == 3))
            ost = ostage.tile([128, 512], f32, name="ost", tag="ost")
            nc.vector.tensor_copy(ost[0:rows, :], po[0:rows, :])
            eng = nc.sync if mb % 2 == 0 else nc.scalar
            eng.dma_start(out_ap[t0:t0 + rows, :], ost[0:rows, :])


# ----------------------------------------------------------------------------
# module build + run
# ----------------------------------------------------------------------------
_CACHE = {}


def _get_module():
    if "nc" in _CACHE:
        return _CACHE["nc"], _CACHE["plans"], _CACHE["mask_tiles"]
    import concourse.tile as tile
    from concourse import bacc, mybir

    plans, (maskcat, mask_offs) = build_block_plan()

    nc = bacc.Bacc("TRN2", target_bir_lowering=False, debug=False,
                   enable_asserts=False, num_devices=NCORES)
    f32 = mybir.dt.float32
    bf16 = mybir.dt.bfloat16

    def din(name, shape, dt=f32):
        return nc.dram_tensor(name, list(shape), dt, kind="ExternalInput").ap()

    ins = dict(
        xT=din("xT", (EMBED, T)),
        wq=din("wq", (EMBED, EMBED)), wk=din("wk", (EMBED, EMBED)),
        wv=din("wv", (EMBED, EMBED)),
        wp=din("wp", (EMBED, EMBED), bf16),
        wqml=din("wqml", (EMBED, 128)), wkml=din("wkml", (EMBED, 128)),
        smalls=din("smalls", (128, 30)),
        masks=din("masks", (128, maskcat.shape[1]), bf16),
        ones10=din("ones10", (128, NSM), bf16),
    )
    out_ap = nc.dram_tensor("out_p", [T, EMBED], f32, kind="ExternalOutput").ap()

    with tile.TileContext(nc) as tc:
        emit_kernel(tc, ins, out_ap, plans, (maskcat.shape[1], mask_offs))
    nc.compile()

    _CACHE.update(nc=nc, plans=plans, mask_tiles=maskcat)
    return nc, plans, maskcat


def build_in_maps(inputs):
    """Per-core input maps; weights/masks prepped once and shared."""
    nc, plans, maskcat = _get_module()
    x = inputs["x"].astype(np.float32)
    cond = np.asarray(inputs["cond_mask"]).astype(np.int32)
    B = x.shape[0]
    assert B == NCORES, f"expected B={NCORES}, got {B}"

    perm, _ = build_perm()
    shared = prep_shared_inputs(inputs)
    mk = np.zeros((128, maskcat.shape[1]), dtype=np.float32)
    mk[:maskcat.shape[0]] = maskcat
    shared["masks"] = mk.astype(BF16)

    in_maps = []
    bias_cache = {}
    for b in range(B):
        ci = dict(shared)
        ci["xT"] = np.ascontiguousarray(x[b][perm].T)
        cb = int(cond[b])
        if cb not in bias_cache:
            sm = shared["smalls"].copy()
            sm[:, 10:20] = biascols_for(inputs, cb)
            bias_cache[cb] = sm
        ci["smalls"] = bias_cache[cb]
        in_maps.append(ci)
    return nc, in_maps


def kernel(**inputs):
    from concourse import bass_utils

    inputs = {k: np.asarray(v) for k, v in inputs.items()}
    nc, in_maps = build_in_maps(inputs)
    res = bass_utils.run_bass_kernel_spmd(nc, in_maps, core_ids=list(range(NCORES)))
    _CACHE["last_results"] = res

    _, inv = build_perm()
    shift = host_const_shift(inputs)
    B = inputs["x"].shape[0]
    out = np.empty((B, T, EMBED), dtype=np.float32)
    for b in range(B):
        out[b] = res.results[b]["out_p"][inv] + shift
    return out
